# revision 4
# baseline (speedup 1.0000x reference)
"""LSTM encoder kernel for Trainium2 (8 NeuronCores, data-parallel over batch).

Host-side prep folds the embedding lookup + input projection into a single
120-row table XC (30 letter tokens x 4 state tokens), gate-reordered so each
H-quarter's PSUM bank pair holds [g|i|f|o]. Per step the device does:
  gates_q = sum_k hT[k].T @ W[k]   (PE, PSUM accumulate; h is the stationary)
  gates_q += xp_q                  (DVE; xp rows gathered from XC by index)
  c, h elementwise on ACT/DVE; h fed back via blocked bf16 transpose-DMA.
"""

from contextlib import ExitStack

import ml_dtypes
import numpy as np

import concourse.bacc as bacc
import concourse.bass as bass
import concourse.mybir as mybir
import concourse.tile as tile
from concourse.bass_utils import run_bass_kernel_spmd

F32 = mybir.dt.float32
BF16 = mybir.dt.bfloat16
I32 = mybir.dt.int32

B, S, E, H = 256, 256, 256, 1024
NCORES = 8
BL = B // NCORES          # 32 batch rows per core
NK = H // 128             # 8 contraction tiles
NQ = 4                    # H quarters
QH = H // NQ              # 256 h-units per quarter
GATE_BLOCKS = [2, 0, 1, 3]  # our order [g, i, f, o] -> pytorch row blocks i,f,g,o

XP_MODE = "dve"           # "dve": indirect-DMA gather + DVE add; "pe": matmul inject

_cache = {}


def _build(steps: int, xp_mode: str = XP_MODE):
    nc = bacc.Bacc("TRN2", target_bir_lowering=False, debug=False, enable_asserts=True)

    w_dram = nc.dram_tensor("W", [H, 4 * H], BF16, kind="ExternalInput")
    xc_dram = nc.dram_tensor("XC", [120, 4 * H], BF16, kind="ExternalInput")
    if xp_mode == "dve":
        idx_dram = nc.dram_tensor("IDX", [BL, steps], I32, kind="ExternalInput")
    else:
        oh_dram = nc.dram_tensor("OH", [120, BL * steps], BF16, kind="ExternalInput")
    hid_dram = nc.dram_tensor("hid", [BL, steps, H], F32, kind="ExternalOutput")
    cell_dram = nc.dram_tensor("cell", [BL, steps, H], F32, kind="ExternalOutput")

    Tanh = mybir.ActivationFunctionType.Tanh
    Sigmoid = mybir.ActivationFunctionType.Sigmoid

    with tile.TileContext(nc) as tc, ExitStack() as ctx:
        resident = ctx.enter_context(tc.tile_pool(name="resident", bufs=1))
        psum_pool = ctx.enter_context(tc.tile_pool(name="psum", bufs=4, space="PSUM"))
        act_pool = ctx.enter_context(tc.tile_pool(name="act", bufs=3))
        h_pool = ctx.enter_context(tc.tile_pool(name="h", bufs=3))
        xp_pool = ctx.enter_context(tc.tile_pool(name="xp", bufs=4))

        w_sb = resident.tile([128, NK, 4 * H], BF16)
        nc.sync.dma_start(w_sb[:], w_dram.ap().rearrange("(k p) n -> p k n", p=128))
        if xp_mode == "dve":
            idx_sb = resident.tile([BL, steps], I32)
            nc.sync.dma_start(idx_sb[:], idx_dram[:])
        else:
            xc_sb = resident.tile([120, 4 * H], BF16)
            nc.sync.dma_start(xc_sb[:], xc_dram[:])
            oh_sb = resident.tile([120, BL * steps], BF16)
            nc.sync.dma_start(oh_sb[:], oh_dram[:])

        # rotating state buffers (3-deep c so the cell-store DMA never stalls
        # the c-update two steps later)
        c_st = [resident.tile([BL, H], F32, name=f"c{i}") for i in range(3)]
        hT_st = [resident.tile([128, NK, BL], BF16, name=f"hT{i}") for i in range(2)]

        xp_tiles = {}

        def issue_xp_gather(t):
            if xp_mode != "dve" or t >= steps:
                return
            xp = xp_pool.tile([BL, 4 * H], BF16, tag="xp", name=f"xp{t}")
            nc.gpsimd.indirect_dma_start(
                out=xp[:], out_offset=None, in_=xc_dram[:],
                in_offset=bass.IndirectOffsetOnAxis(ap=idx_sb[:, t:t + 1], axis=0),
            )
            xp_tiles[t] = xp

        for t in range(min(3, steps)):
            issue_xp_gather(t)

        for t in range(steps):
            c_old = c_st[(t + 2) % 3]   # written at t-1
            c_new = c_st[t % 3]
            hT_cur = hT_st[t % 2]
            hT_nxt = hT_st[(t + 1) % 2]

            hbf = h_pool.tile([BL, H], BF16, tag="hbf", name=f"hbf{t}")

            if xp_mode == "dve":
                xp = xp_tiles.pop(t)
                issue_xp_gather(t + 3)

            for q in range(NQ):
                qc = 1024 * q
                qs = slice(QH * q, QH * (q + 1))

                if xp_mode == "pe":
                    oh_t = oh_sb[:, BL * t:BL * (t + 1)]
                    ps = psum_pool.tile([BL, 1024], F32, tag="ps", name=f"ps{t}_{q}")
                    nc.tensor.matmul(ps[:, 0:512], oh_t, xc_sb[:, qc:qc + 512],
                                     start=True, stop=(t == 0))
                    nc.tensor.matmul(ps[:, 512:1024], oh_t,
                                     xc_sb[:, qc + 512:qc + 1024],
                                     start=True, stop=(t == 0))
                    if t > 0:
                        for k in range(NK):
                            nc.tensor.matmul(ps[:, 0:512], hT_cur[:, k],
                                             w_sb[:, k, qc:qc + 512],
                                             start=False, stop=(k == NK - 1))
                            nc.tensor.matmul(ps[:, 512:1024], hT_cur[:, k],
                                             w_sb[:, k, qc + 512:qc + 1024],
                                             start=False, stop=(k == NK - 1))
                    gate_src = ps
                else:
                    if t > 0:
                        ps = psum_pool.tile([BL, 1024], F32, tag="ps", name=f"ps{t}_{q}")
                        for k in range(NK):
                            nc.tensor.matmul(ps[:, 0:512], hT_cur[:, k],
                                             w_sb[:, k, qc:qc + 512],
                                             start=(k == 0), stop=(k == NK - 1))
                            nc.tensor.matmul(ps[:, 512:1024], hT_cur[:, k],
                                             w_sb[:, k, qc + 512:qc + 1024],
                                             start=(k == 0), stop=(k == NK - 1))
                        gate_src = act_pool.tile([BL, 1024], F32, tag="gates",
                                                 name=f"gates{t}_{q}")
                        nc.vector.tensor_add(gate_src[:], ps[:], xp[:, qc:qc + 1024])
                    else:
                        gate_src = xp[:, qc:qc + 1024]

                # quarter layout: [g(256) | i(256) | f(256) | o(256)]
                g_t = act_pool.tile([BL, QH], F32, tag="g", name=f"g{t}_{q}")
                nc.scalar.activation(g_t[:], gate_src[:, 0:256], Tanh)
                ifo = act_pool.tile([BL, 3 * QH], F32, tag="ifo", name=f"ifo{t}_{q}")
                nc.scalar.activation(ifo[:], gate_src[:, 256:1024], Sigmoid)

                if t == 0:
                    nc.vector.tensor_mul(c_new[:, qs], ifo[:, 0:256], g_t[:])
                else:
                    t1 = act_pool.tile([BL, QH], F32, tag="t1", name=f"t1{t}_{q}")
                    nc.vector.tensor_mul(t1[:], ifo[:, 0:256], g_t[:])
                    nc.vector.tensor_mul(c_new[:, qs], ifo[:, 256:512], c_old[:, qs])
                    nc.vector.tensor_add(c_new[:, qs], c_new[:, qs], t1[:])

                th = act_pool.tile([BL, QH], F32, tag="th", name=f"th{t}_{q}")
                nc.scalar.activation(th[:], c_new[:, qs], Tanh)
                nc.vector.tensor_mul(hbf[:, qs], ifo[:, 512:768], th[:])

                if t < steps - 1:
                    nc.sync.dma_start(hT_nxt[:, 2 * q], hbf[:, QH * q:QH * q + 128],
                                      transpose=True)
                    nc.sync.dma_start(hT_nxt[:, 2 * q + 1],
                                      hbf[:, QH * q + 128:QH * q + 256],
                                      transpose=True)

            nc.gpsimd.dma_start(hid_dram[:, t, :], hbf[:])      # bf16 -> f32 cast DMA
            nc.scalar.dma_start(cell_dram[:, t, :], c_new[:])   # f32, HWDGE (ACT ring)

    nc.compile()
    return nc


def _host_prep(letter_seq, state_seq, letter_emb, state_emb, W_ih, W_hh, b_ih, b_hh,
               steps: int, xp_mode: str = XP_MODE):
    letter_seq = np.asarray(letter_seq)
    state_seq = np.asarray(state_seq)
    letter_emb = np.asarray(letter_emb, dtype=np.float32)
    state_emb = np.asarray(state_emb, dtype=np.float32)
    W_ih = np.asarray(W_ih, dtype=np.float32)
    W_hh = np.asarray(W_hh, dtype=np.float32)
    b_ih = np.asarray(b_ih, dtype=np.float32)
    b_hh = np.asarray(b_hh, dtype=np.float32)

    # column permutation: new col j = q*1024 + blk*256 + r  ->  orig 4H row
    q_idx = np.arange(4 * H) // 1024
    blk = (np.arange(4 * H) % 1024) // QH
    r = np.arange(4 * H) % QH
    colmap = np.array(GATE_BLOCKS)[blk] * H + q_idx * QH + r  # [4H]

    Wp = np.ascontiguousarray(W_hh[colmap, :].T).astype(ml_dtypes.bfloat16)  # [H, 4H]

    XL = letter_emb @ W_ih[:, :E].T                            # [30, 4H]
    XS = state_emb @ W_ih[:, E:].T                             # [4, 4H]
    bias = b_ih + b_hh
    XC = (XL[:, None, :] + XS[None, :, :] + bias).reshape(120, 4 * H)
    XC = np.ascontiguousarray(XC[:, colmap]).astype(ml_dtypes.bfloat16)  # [120, 4H]

    idx = (letter_seq.astype(np.int64) * 4 + state_seq.astype(np.int64))  # [B, S]
    in_maps = []
    for c in range(NCORES):
        idx_c = idx[BL * c:BL * (c + 1), :steps]               # [BL, steps]
        m = {"W": Wp, "XC": XC}
        if xp_mode == "dve":
            m["IDX"] = np.ascontiguousarray(idx_c).astype(np.int32)
        else:
            oh = np.zeros((120, BL * steps), dtype=ml_dtypes.bfloat16)
            cols = np.arange(BL * steps)
            oh[idx_c.T.reshape(-1), cols] = 1.0                # col = t*BL + b
            m["OH"] = oh
        in_maps.append(m)
    return in_maps


def kernel(letter_seq, state_seq, letter_emb, state_emb, W_ih, W_hh, b_ih, b_hh,
           steps: int = S):
    key = (steps, XP_MODE)
    if key not in _cache:
        _cache[key] = _build(steps, XP_MODE)
    nc = _cache[key]

    in_maps = _host_prep(letter_seq, state_seq, letter_emb, state_emb,
                         W_ih, W_hh, b_ih, b_hh, steps, XP_MODE)
    res = run_bass_kernel_spmd(nc, in_maps, core_ids=list(range(NCORES)))

    hidden = np.concatenate([res.results[c]["hid"] for c in range(NCORES)], axis=0)
    cell = np.concatenate([res.results[c]["cell"] for c in range(NCORES)], axis=0)
    return hidden, cell


# revision 5
# speedup vs baseline: 1.2498x; 1.2498x over previous
"""LSTM encoder kernel for Trainium2 (8 NeuronCores, data-parallel over batch).

Host-side prep folds the embedding lookup + input projection into a single
120-row table XC (30 letter tokens x 4 state tokens), gate-reordered so each
H-quarter's PSUM bank pair holds [g|i|f|o]. Per step the device does:
  gates_q = sum_k hT[k].T @ W[k]   (PE, PSUM accumulate; h is the stationary)
  gates_q += xp_q                  (DVE; xp rows gathered from XC by index)
  c, h elementwise on ACT/DVE; h fed back via blocked bf16 transpose-DMA.
"""

from contextlib import ExitStack

import ml_dtypes
import numpy as np

import concourse.bacc as bacc
import concourse.bass as bass
import concourse.mybir as mybir
import concourse.tile as tile
from concourse.bass_utils import run_bass_kernel_spmd

F32 = mybir.dt.float32
BF16 = mybir.dt.bfloat16
I32 = mybir.dt.int32

B, S, E, H = 256, 256, 256, 1024
NCORES = 8
BL = B // NCORES          # 32 batch rows per core
NK = H // 128             # 8 contraction tiles
NQ = 4                    # H quarters
QH = H // NQ              # 256 h-units per quarter
GATE_BLOCKS = [2, 0, 1, 3]  # our order [g, i, f, o] -> pytorch row blocks i,f,g,o

XP_MODE = "dve"           # "dve": indirect-DMA gather + DVE add; "pe": matmul inject

_cache = {}


def _build(steps: int, xp_mode: str = XP_MODE):
    nc = bacc.Bacc("TRN2", target_bir_lowering=False, debug=False, enable_asserts=True)

    w_dram = nc.dram_tensor("W", [H, 4 * H], BF16, kind="ExternalInput")
    xc_dram = nc.dram_tensor("XC", [120, 4 * H], BF16, kind="ExternalInput")
    if xp_mode == "dve":
        idx_dram = nc.dram_tensor("IDX", [BL, steps], I32, kind="ExternalInput")
    else:
        oh_dram = nc.dram_tensor("OH", [120, BL * steps], BF16, kind="ExternalInput")
    hid_dram = nc.dram_tensor("hid", [BL, steps, H], F32, kind="ExternalOutput")
    cell_dram = nc.dram_tensor("cell", [BL, steps, H], F32, kind="ExternalOutput")

    Tanh = mybir.ActivationFunctionType.Tanh
    Sigmoid = mybir.ActivationFunctionType.Sigmoid

    with tile.TileContext(nc) as tc, ExitStack() as ctx:
        resident = ctx.enter_context(tc.tile_pool(name="resident", bufs=1))
        psum_pool = ctx.enter_context(tc.tile_pool(name="psum", bufs=1, space="PSUM"))
        act_pool = ctx.enter_context(tc.tile_pool(name="act", bufs=3))
        h_pool = ctx.enter_context(tc.tile_pool(name="h", bufs=3))
        xp_pool = ctx.enter_context(tc.tile_pool(name="xp", bufs=4))

        w_sb = resident.tile([128, NK, 4 * H], BF16)
        nc.sync.dma_start(w_sb[:], w_dram.ap().rearrange("(k p) n -> p k n", p=128))
        if xp_mode == "dve":
            idx_sb = resident.tile([BL, steps], I32)
            nc.sync.dma_start(idx_sb[:], idx_dram[:])
        else:
            xc_sb = resident.tile([120, 4 * H], BF16)
            nc.sync.dma_start(xc_sb[:], xc_dram[:])
            oh_sb = resident.tile([120, BL * steps], BF16)
            nc.sync.dma_start(oh_sb[:], oh_dram[:])

        # persistent per-quarter PSUM tiles: step t+1's quarter-q matmuls wait
        # only on step t's quarter-q gates-add (deterministic pipelining;
        # the pool's LIFO slot reuse would chain q0 onto q3's late free)
        ps_q = [psum_pool.tile([BL, 1024], F32, name=f"psq{i}") for i in range(NQ)]

        # rotating state buffers (3-deep c so the cell-store DMA never stalls
        # the c-update two steps later)
        c_st = [resident.tile([BL, H], F32, name=f"c{i}") for i in range(3)]
        hT_st = [resident.tile([128, NK, BL], BF16, name=f"hT{i}") for i in range(2)]

        xp_tiles = {}

        def issue_xp_gather(t):
            if xp_mode != "dve" or t >= steps:
                return
            xp = xp_pool.tile([BL, 4 * H], BF16, tag="xp", name=f"xp{t}")
            nc.gpsimd.indirect_dma_start(
                out=xp[:], out_offset=None, in_=xc_dram[:],
                in_offset=bass.IndirectOffsetOnAxis(ap=idx_sb[:, t:t + 1], axis=0),
            )
            xp_tiles[t] = xp

        for t in range(min(3, steps)):
            issue_xp_gather(t)

        for t in range(steps):
            c_old = c_st[(t + 2) % 3]   # written at t-1
            c_new = c_st[t % 3]
            hT_cur = hT_st[t % 2]
            hT_nxt = hT_st[(t + 1) % 2]

            hbf = h_pool.tile([BL, H], BF16, tag="hbf", name=f"hbf{t}")

            if xp_mode == "dve":
                xp = xp_tiles.pop(t)
                issue_xp_gather(t + 3)

            for q in range(NQ):
                qc = 1024 * q
                qs = slice(QH * q, QH * (q + 1))

                if xp_mode == "pe":
                    oh_t = oh_sb[:, BL * t:BL * (t + 1)]
                    ps = ps_q[q]
                    nc.tensor.matmul(ps[:, 0:512], oh_t, xc_sb[:, qc:qc + 512],
                                     start=True, stop=(t == 0))
                    nc.tensor.matmul(ps[:, 512:1024], oh_t,
                                     xc_sb[:, qc + 512:qc + 1024],
                                     start=True, stop=(t == 0))
                    if t > 0:
                        for k in range(NK):
                            nc.tensor.matmul(ps[:, 0:512], hT_cur[:, k],
                                             w_sb[:, k, qc:qc + 512],
                                             start=False, stop=(k == NK - 1))
                            nc.tensor.matmul(ps[:, 512:1024], hT_cur[:, k],
                                             w_sb[:, k, qc + 512:qc + 1024],
                                             start=False, stop=(k == NK - 1))
                    gate_src = ps
                else:
                    if t > 0:
                        ps = ps_q[q]
                        for k in range(NK):
                            nc.tensor.matmul(ps[:, 0:512], hT_cur[:, k],
                                             w_sb[:, k, qc:qc + 512],
                                             start=(k == 0), stop=(k == NK - 1))
                            nc.tensor.matmul(ps[:, 512:1024], hT_cur[:, k],
                                             w_sb[:, k, qc + 512:qc + 1024],
                                             start=(k == 0), stop=(k == NK - 1))
                        gate_src = act_pool.tile([BL, 1024], F32, tag="gates",
                                                 name=f"gates{t}_{q}")
                        nc.vector.tensor_add(gate_src[:], ps[:], xp[:, qc:qc + 1024])
                    else:
                        gate_src = xp[:, qc:qc + 1024]

                # quarter layout: [g(256) | i(256) | f(256) | o(256)]
                g_t = act_pool.tile([BL, QH], F32, tag="g", name=f"g{t}_{q}")
                nc.scalar.activation(g_t[:], gate_src[:, 0:256], Tanh)
                ifo = act_pool.tile([BL, 3 * QH], F32, tag="ifo", name=f"ifo{t}_{q}")
                nc.scalar.activation(ifo[:], gate_src[:, 256:1024], Sigmoid)

                if t == 0:
                    nc.vector.tensor_mul(c_new[:, qs], ifo[:, 0:256], g_t[:])
                else:
                    t1 = act_pool.tile([BL, QH], F32, tag="t1", name=f"t1{t}_{q}")
                    nc.vector.tensor_mul(t1[:], ifo[:, 0:256], g_t[:])
                    nc.vector.tensor_mul(c_new[:, qs], ifo[:, 256:512], c_old[:, qs])
                    nc.vector.tensor_add(c_new[:, qs], c_new[:, qs], t1[:])

                th = act_pool.tile([BL, QH], F32, tag="th", name=f"th{t}_{q}")
                nc.scalar.activation(th[:], c_new[:, qs], Tanh)
                nc.vector.tensor_mul(hbf[:, qs], ifo[:, 512:768], th[:])

                if t < steps - 1:
                    nc.sync.dma_start(hT_nxt[:, 2 * q], hbf[:, QH * q:QH * q + 128],
                                      transpose=True)
                    nc.sync.dma_start(hT_nxt[:, 2 * q + 1],
                                      hbf[:, QH * q + 128:QH * q + 256],
                                      transpose=True)

            nc.gpsimd.dma_start(hid_dram[:, t, :], hbf[:])      # bf16 -> f32 cast DMA
            nc.scalar.dma_start(cell_dram[:, t, :], c_new[:])   # f32, HWDGE (ACT ring)

    nc.compile()
    return nc


def _host_prep(letter_seq, state_seq, letter_emb, state_emb, W_ih, W_hh, b_ih, b_hh,
               steps: int, xp_mode: str = XP_MODE):
    letter_seq = np.asarray(letter_seq)
    state_seq = np.asarray(state_seq)
    letter_emb = np.asarray(letter_emb, dtype=np.float32)
    state_emb = np.asarray(state_emb, dtype=np.float32)
    W_ih = np.asarray(W_ih, dtype=np.float32)
    W_hh = np.asarray(W_hh, dtype=np.float32)
    b_ih = np.asarray(b_ih, dtype=np.float32)
    b_hh = np.asarray(b_hh, dtype=np.float32)

    # column permutation: new col j = q*1024 + blk*256 + r  ->  orig 4H row
    q_idx = np.arange(4 * H) // 1024
    blk = (np.arange(4 * H) % 1024) // QH
    r = np.arange(4 * H) % QH
    colmap = np.array(GATE_BLOCKS)[blk] * H + q_idx * QH + r  # [4H]

    Wp = np.ascontiguousarray(W_hh[colmap, :].T).astype(ml_dtypes.bfloat16)  # [H, 4H]

    XL = letter_emb @ W_ih[:, :E].T                            # [30, 4H]
    XS = state_emb @ W_ih[:, E:].T                             # [4, 4H]
    bias = b_ih + b_hh
    XC = (XL[:, None, :] + XS[None, :, :] + bias).reshape(120, 4 * H)
    XC = np.ascontiguousarray(XC[:, colmap]).astype(ml_dtypes.bfloat16)  # [120, 4H]

    idx = (letter_seq.astype(np.int64) * 4 + state_seq.astype(np.int64))  # [B, S]
    in_maps = []
    for c in range(NCORES):
        idx_c = idx[BL * c:BL * (c + 1), :steps]               # [BL, steps]
        m = {"W": Wp, "XC": XC}
        if xp_mode == "dve":
            m["IDX"] = np.ascontiguousarray(idx_c).astype(np.int32)
        else:
            oh = np.zeros((120, BL * steps), dtype=ml_dtypes.bfloat16)
            cols = np.arange(BL * steps)
            oh[idx_c.T.reshape(-1), cols] = 1.0                # col = t*BL + b
            m["OH"] = oh
        in_maps.append(m)
    return in_maps


def kernel(letter_seq, state_seq, letter_emb, state_emb, W_ih, W_hh, b_ih, b_hh,
           steps: int = S):
    key = (steps, XP_MODE)
    if key not in _cache:
        _cache[key] = _build(steps, XP_MODE)
    nc = _cache[key]

    in_maps = _host_prep(letter_seq, state_seq, letter_emb, state_emb,
                         W_ih, W_hh, b_ih, b_hh, steps, XP_MODE)
    res = run_bass_kernel_spmd(nc, in_maps, core_ids=list(range(NCORES)))

    hidden = np.concatenate([res.results[c]["hid"] for c in range(NCORES)], axis=0)
    cell = np.concatenate([res.results[c]["cell"] for c in range(NCORES)], axis=0)
    return hidden, cell
